# revision 1
# baseline (speedup 1.0000x reference)
"""Local (sliding-window w=2) attention, B=4 S=2048 H=1024, on 8 trn2 cores.

Strategy: sequence-parallel. Each core owns half of one batch's sequence
(1024 tokens) plus a 2-token halo on each side (ext = 1028 tokens).
Per core:
  QT/KT = (Wq|Wk)^T-style projections in feature-major layout [H, T]
  (f32r matmuls, bf16 outputs for the score path), V computed per
  attention block in token-major layout [T, H] (f32r), then 9 q-blocks
  of 124 queries: band scores (window 128), masked softmax, P transpose
  on PE, P@V, bias add, DMA out.
Host side: shard + transpose x, tf32-round PE operands, build masks.
"""

import sys

sys.path.insert(0, "/opt/trn_rl_repo")

import numpy as np

import concourse.bass as bass  # noqa: F401  (bass must import before tile)
import concourse.mybir as mybir
import concourse.tile as tile
from concourse import bacc
from concourse.bass_utils import run_bass_kernel_spmd

F32 = mybir.dt.float32
F32R = mybir.dt.float32r
BF16 = mybir.dt.bfloat16

B, S, H = 4, 2048, 1024
WCTX = 2
NCORES = 8
SHARD = S // 2  # tokens per core
EXT = SHARD + 2 * WCTX  # 1028
P = 128
QB = 124  # queries per attention block
WIN = QB + 2 * WCTX  # 128 = key window per block
NBLK = (SHARD + QB - 1) // QB  # 9
HC = H // P  # 8 feature chunks
SCALE = 1.0 / np.sqrt(np.float32(H))

_prog_cache = {}


def _build_program():
    nc = bacc.Bacc("TRN2", target_bir_lowering=False, debug=False)
    xt_d = nc.dram_tensor("xt", [H, EXT], F32R, kind="ExternalInput").ap()
    wq_d = nc.dram_tensor("wq", [H, H], F32R, kind="ExternalInput").ap()
    wk_d = nc.dram_tensor("wk", [H, H], F32R, kind="ExternalInput").ap()
    wv_d = nc.dram_tensor("wv", [H, H], F32R, kind="ExternalInput").ap()
    bq_d = nc.dram_tensor("bq_c", [P, HC], F32, kind="ExternalInput").ap()
    bk_d = nc.dram_tensor("bk_c", [P, HC], F32, kind="ExternalInput").ap()
    bv_d = nc.dram_tensor("bv_b", [P, H], F32, kind="ExternalInput").ap()
    id_d = nc.dram_tensor("ident", [P, P], F32R, kind="ExternalInput").ap()
    mk_d = nc.dram_tensor("mask", [NBLK, QB, WIN], F32, kind="ExternalInput").ap()
    out_d = nc.dram_tensor("out", [SHARD, H], F32, kind="ExternalOutput").ap()

    xt_r = xt_d.rearrange("(hc p) t -> p hc t", p=P)
    wq_r = wq_d.rearrange("(hc p) j -> p hc j", p=P)
    wk_r = wk_d.rearrange("(hc p) j -> p hc j", p=P)
    wv_r = wv_d.rearrange("(hc p) j -> p hc j", p=P)
    mk_r = mk_d.rearrange("b q c -> q b c")

    with tile.TileContext(nc) as tc:
        with (
            tc.tile_pool(name="persist", bufs=1) as pers,
            tc.tile_pool(name="wpool", bufs=2) as wpool,
            tc.tile_pool(name="vpool", bufs=2) as vpool,
            tc.tile_pool(name="spool", bufs=2) as spool,
            tc.tile_pool(name="opool", bufs=2) as opool,
            tc.tile_pool(name="pproj", bufs=2, space="PSUM") as pproj,
            tc.tile_pool(name="patt", bufs=2, space="PSUM") as patt,
            tc.tile_pool(name="pout", bufs=1, space="PSUM") as pout,
        ):
            # ---- constants ----
            ident = pers.tile([P, P], F32R)
            nc.sync.dma_start(ident[:], id_d)
            bqc = pers.tile([P, HC], F32)
            nc.sync.dma_start(bqc[:], bq_d)
            bkc = pers.tile([P, HC], F32)
            nc.sync.dma_start(bkc[:], bk_d)
            bvb = pers.tile([P, H], F32)
            nc.sync.dma_start(bvb[:], bv_d)
            maskt = pers.tile([QB, NBLK, WIN], F32)
            nc.sync.dma_start(maskt[:], mk_r)

            # ---- weights (j-chunk-major DMA so compute starts early) ----
            wq_sb = wpool.tile([P, HC, H], F32R, tag="w")
            for jc in range(HC):
                for hc in range(HC):
                    nc.sync.dma_start(
                        wq_sb[:, hc, jc * P : (jc + 1) * P],
                        wq_r[:, hc, jc * P : (jc + 1) * P],
                    )
            # ---- x^T, in halves per feature chunk ----
            xt_sb = pers.tile([P, HC, EXT], F32R)
            for hc in range(HC):
                for t0, t1 in ((0, EXT // 2), (EXT // 2, EXT)):
                    nc.sync.dma_start(xt_sb[:, hc, t0:t1], xt_r[:, hc, t0:t1])

            wk_sb = wpool.tile([P, HC, H], F32R, tag="w")
            for jc in range(HC):
                for hc in range(HC):
                    nc.sync.dma_start(
                        wk_sb[:, hc, jc * P : (jc + 1) * P],
                        wk_r[:, hc, jc * P : (jc + 1) * P],
                    )
            wv_sb = wpool.tile([P, HC, H], F32R, tag="w")
            for hc in range(HC):
                nc.sync.dma_start(wv_sb[:, hc, :], wv_r[:, hc, :])

            # ---- Q^T projection: [j, t] bf16, owned tokens ext [2, 1026) ----
            qt_sb = pers.tile([P, HC, SHARD], BF16)
            for jc in range(HC):
                for t in range(2):
                    ps = pproj.tile([P, 512], F32, tag="proj")
                    for hc in range(HC):
                        nc.tensor.matmul(
                            ps[:],
                            wq_sb[:, hc, jc * P : (jc + 1) * P],
                            xt_sb[:, hc, 2 + 512 * t : 2 + 512 * (t + 1)],
                            start=(hc == 0),
                            stop=(hc == HC - 1),
                        )
                    nc.vector.tensor_scalar_add(
                        qt_sb[:, jc, 512 * t : 512 * (t + 1)],
                        ps[:],
                        bqc[:, jc : jc + 1],
                    )

            # ---- K^T projection: [j, t] bf16, all ext tokens [0, 1028) ----
            kt_sb = pers.tile([P, HC, EXT], BF16)
            for jc in range(HC):
                for t0, t1 in ((0, 512), (512, 1024), (1024, EXT)):
                    ps = pproj.tile([P, 512], F32, tag="proj")
                    n = t1 - t0
                    for hc in range(HC):
                        nc.tensor.matmul(
                            ps[:, :n],
                            wk_sb[:, hc, jc * P : (jc + 1) * P],
                            xt_sb[:, hc, t0:t1],
                            start=(hc == 0),
                            stop=(hc == HC - 1),
                        )
                    nc.vector.tensor_scalar_add(
                        kt_sb[:, jc, t0:t1], ps[:, :n], bkc[:, jc : jc + 1]
                    )

            # ---- attention blocks ----
            for b in range(NBLK):
                q0 = QB * b
                qb = min(QB, SHARD - q0)
                w = qb + 2 * WCTX
                e0 = q0  # ext index of window start

                # V for this block's window, token-major [w, H], f32r
                vb = vpool.tile([P, H], F32R, tag="vblk")
                for n in range(2):
                    psv = pproj.tile([P, 512], F32, tag="proj")
                    for hc in range(HC):
                        nc.tensor.matmul(
                            psv[:w, :],
                            xt_sb[:, hc, e0 : e0 + w],
                            wv_sb[:, hc, 512 * n : 512 * (n + 1)],
                            start=(hc == 0),
                            stop=(hc == HC - 1),
                        )
                    nc.vector.tensor_copy(vb[:w, 512 * n : 512 * (n + 1)], psv[:w, :])

                # scores [qb, w] = QT_blk^T @ KT_window  (bf16 operands)
                pss = patt.tile([QB, WIN], F32, tag="ps")
                for jc in range(HC):
                    nc.tensor.matmul(
                        pss[:qb, :w],
                        qt_sb[:, jc, q0 : q0 + qb],
                        kt_sb[:, jc, e0 : e0 + w],
                        start=(jc == 0),
                        stop=(jc == HC - 1),
                    )

                # masked softmax over the window
                sm = spool.tile([QB, WIN], F32, tag="sm")
                nc.vector.tensor_tensor(
                    sm[:qb, :w], pss[:qb, :w], maskt[:qb, b, :w], op=mybir.AluOpType.add
                )
                pexp = spool.tile([QB, WIN], F32, tag="pexp")
                rsum = spool.tile([QB, 1], F32, tag="rsum")
                nc.scalar.activation(
                    pexp[:qb, :w],
                    sm[:qb, :w],
                    mybir.ActivationFunctionType.Exp,
                    bias=0.0,
                    scale=float(SCALE),
                    accum_out=rsum[:qb],
                )
                rcp = spool.tile([QB, 1], F32, tag="rcp")
                nc.vector.reciprocal(rcp[:qb], rsum[:qb])
                pn = spool.tile([QB, WIN], F32R, tag="pn")
                nc.vector.tensor_scalar_mul(pn[:qb, :w], pexp[:qb, :w], rcp[:qb])

                # P^T via PE transpose, then to SBUF
                pst = patt.tile([WIN, QB], F32R, tag="pt")
                nc.tensor.transpose(pst[:w, :qb], pn[:qb, :w], ident[:qb, :qb])
                pts = spool.tile([WIN, QB], F32R, tag="pts")
                nc.vector.tensor_copy(pts[:w, :qb], pst[:w, :qb])

                # O = P @ V  [qb, H]
                pso = pout.tile([QB, H], F32, tag="po")
                for n in range(2):
                    nc.tensor.matmul(
                        pso[:qb, 512 * n : 512 * (n + 1)],
                        pts[:w, :qb],
                        vb[:w, 512 * n : 512 * (n + 1)],
                        start=True,
                        stop=True,
                    )
                ob = opool.tile([QB, H], F32, tag="ob")
                nc.vector.tensor_tensor(
                    ob[:qb, :], pso[:qb, :], bvb[:qb, :], op=mybir.AluOpType.add
                )
                nc.sync.dma_start(out_d[q0 : q0 + qb, :], ob[:qb, :])

    nc.compile()
    return nc


def _tf32_round(a: np.ndarray) -> np.ndarray:
    u = np.ascontiguousarray(a, dtype=np.float32).view(np.uint32)
    r = (u + np.uint32(0x7FF) + ((u >> np.uint32(12)) & np.uint32(1))) & np.uint32(
        0xFFFFF000
    )
    return r.view(np.float32)


def _build_mask(h: int) -> np.ndarray:
    mask = np.full((NBLK, QB, WIN), -1e30, dtype=np.float32)
    r = np.arange(QB)[:, None]
    c = np.arange(WIN)[None, :]
    band = (c - r >= 0) & (c - r <= 2 * WCTX)
    for b in range(NBLK):
        q0 = QB * b
        qb = min(QB, SHARD - q0)
        gk = h * SHARD + q0 + c - WCTX  # global key token index
        valid = band & (gk >= 0) & (gk < S) & (r < qb) & (c < qb + 2 * WCTX)
        mask[b] = np.where(valid, np.float32(0.0), np.float32(-1e30))
    return mask


def kernel(sequence_output, Wq, bq, Wk, bk, Wv, bv):
    x = np.asarray(sequence_output, dtype=np.float32)
    Wq = np.asarray(Wq, dtype=np.float32)
    Wk = np.asarray(Wk, dtype=np.float32)
    Wv = np.asarray(Wv, dtype=np.float32)
    bq = np.asarray(bq, dtype=np.float32)
    bk = np.asarray(bk, dtype=np.float32)
    bv = np.asarray(bv, dtype=np.float32)

    if "nc" not in _prog_cache:
        _prog_cache["nc"] = _build_program()
    nc = _prog_cache["nc"]

    wq_r = _tf32_round(Wq)
    wk_r = _tf32_round(Wk)
    wv_r = _tf32_round(Wv)
    bq_c = np.ascontiguousarray(bq.reshape(HC, P).T)
    bk_c = np.ascontiguousarray(bk.reshape(HC, P).T)
    bv_b = np.ascontiguousarray(np.broadcast_to(bv, (P, H)))
    ident = np.eye(P, dtype=np.float32)
    masks = [_build_mask(0), _build_mask(1)]

    # pad each sequence with WCTX zero rows on both ends, slice ext windows
    xp = np.zeros((B, S + 2 * WCTX, H), dtype=np.float32)
    xp[:, WCTX : WCTX + S] = x

    in_maps = []
    for c in range(NCORES):
        bidx, h = divmod(c, 2)
        ext = xp[bidx, h * SHARD : h * SHARD + EXT]  # [EXT, H]
        xt = _tf32_round(np.ascontiguousarray(ext.T))  # [H, EXT]
        in_maps.append(
            {
                "xt": xt,
                "wq": wq_r,
                "wk": wk_r,
                "wv": wv_r,
                "bq_c": bq_c,
                "bk_c": bk_c,
                "bv_b": bv_b,
                "ident": ident,
                "mask": masks[h],
            }
        )

    import os

    trace = bool(int(os.environ.get("LK_TRACE", "0")))
    res = run_bass_kernel_spmd(
        nc,
        in_maps,
        core_ids=list(range(NCORES)),
        trace=trace,
        trace_cores=list(range(NCORES)) if trace else None,
    )
    _prog_cache["last_results"] = res

    out = np.empty((B, S, H), dtype=np.float32)
    for c in range(NCORES):
        bidx, h = divmod(c, 2)
        out[bidx, h * SHARD : (h + 1) * SHARD] = res.results[c]["out"]
    return out


# revision 3
# speedup vs baseline: 1.2263x; 1.2263x over previous
"""Local (sliding-window w=2) attention, B=4 S=2048 H=1024, on 8 trn2 cores.

Strategy: sequence-parallel. Each core owns half of one batch's sequence
(1024 tokens) plus a 2-token halo on each side (ext = 1028 tokens).
Per core:
  Q^T/K^T projections in feature-major layout [H, T] (bf16 operands --
  the softmax damps score-path rounding), V computed per attention block
  in token-major layout [T, H] with f32r (tf32-like) matmuls since V
  error hits the output linearly. Then 9 q-blocks of 124 queries: band
  scores (window 128, bf16), masked softmax (ACT exp with fused row-sum),
  P transpose on PE, P@V (f32r), bias add, DMA out.
Host side: shard + transpose x, tf32/bf16-round PE operands, build masks.
"""

import os
import sys

sys.path.insert(0, "/opt/trn_rl_repo")

import ml_dtypes
import numpy as np

import concourse.bass as bass  # noqa: F401  (bass must import before tile)
import concourse.mybir as mybir
import concourse.tile as tile
from concourse import bacc
from concourse.bass_utils import run_bass_kernel_spmd

F32 = mybir.dt.float32
F32R = mybir.dt.float32r
BF16 = mybir.dt.bfloat16

B, S, H = 4, 2048, 1024
WCTX = 2
NCORES = 8
SHARD = S // 2  # tokens per core
EXT = SHARD + 2 * WCTX  # 1028
HEXT = EXT // 2  # 514
P = 128
QB = 124  # queries per attention block
WIN = QB + 2 * WCTX  # 128 = key window per block
NBLK = (SHARD + QB - 1) // QB  # 9
HC = H // P  # 8 feature chunks
SCALE = 1.0 / np.sqrt(np.float32(H))

_prog_cache = {}


def _build_program():
    nc = bacc.Bacc("TRN2", target_bir_lowering=False, debug=False)
    xt_d = nc.dram_tensor("xt", [H, EXT], F32R, kind="ExternalInput").ap()
    wq_d = nc.dram_tensor("wq", [H, H], BF16, kind="ExternalInput").ap()
    wk_d = nc.dram_tensor("wk", [H, H], BF16, kind="ExternalInput").ap()
    wv_d = nc.dram_tensor("wv", [H, H], F32R, kind="ExternalInput").ap()
    bq_d = nc.dram_tensor("bq_c", [P, HC], F32, kind="ExternalInput").ap()
    bk_d = nc.dram_tensor("bk_c", [P, HC], F32, kind="ExternalInput").ap()
    bv_d = nc.dram_tensor("bv_b", [P, H], F32, kind="ExternalInput").ap()
    id_d = nc.dram_tensor("ident", [P, P], F32R, kind="ExternalInput").ap()
    mk_d = nc.dram_tensor("mask", [NBLK, QB, WIN], F32, kind="ExternalInput").ap()
    out_d = nc.dram_tensor("out", [SHARD, H], F32, kind="ExternalOutput").ap()

    xt_r = xt_d.rearrange("(hc p) t -> p hc t", p=P)
    wq_r = wq_d.rearrange("(hc p) j -> p hc j", p=P)
    wk_r = wk_d.rearrange("(hc p) j -> p hc j", p=P)
    wv_r = wv_d.rearrange("(hc p) j -> p hc j", p=P)
    mk_r = mk_d.rearrange("b q c -> q b c")

    with tile.TileContext(nc) as tc:
        with (
            tc.tile_pool(name="persist", bufs=1) as pers,
            tc.tile_pool(name="wpool", bufs=1) as wpool,
            tc.tile_pool(name="vpool", bufs=2) as vpool,
            tc.tile_pool(name="spool", bufs=2) as spool,
            tc.tile_pool(name="opool", bufs=2) as opool,
            tc.tile_pool(name="pproj", bufs=2, space="PSUM") as pproj,
            tc.tile_pool(name="patt", bufs=2, space="PSUM") as patt,
            tc.tile_pool(name="pout", bufs=1, space="PSUM") as pout,
        ):
            # ---- x^T halves per feature chunk (sync/HWDGE), cast to bf16 ----
            xt_sb = pers.tile([P, HC, EXT], F32R)
            xtb = pers.tile([P, HC, EXT], BF16)
            for hc in range(HC):
                for t0, t1 in ((0, HEXT), (HEXT, EXT)):
                    nc.sync.dma_start(xt_sb[:, hc, t0:t1], xt_r[:, hc, t0:t1])
                    nc.vector.tensor_copy(xtb[:, hc, t0:t1], xt_sb[:, hc, t0:t1])

            # ---- weights on gpsimd/SWDGE, j-halves first for early start ----
            wq_sb = wpool.tile([P, HC, H], BF16, tag="w")
            wk_sb = wpool.tile([P, HC, H], BF16, tag="wk")
            for j0, j1 in ((0, 512), (512, H)):
                for hc in range(HC):
                    nc.gpsimd.dma_start(wq_sb[:, hc, j0:j1], wq_r[:, hc, j0:j1])
            for j0, j1 in ((0, 512), (512, H)):
                for hc in range(HC):
                    nc.gpsimd.dma_start(wk_sb[:, hc, j0:j1], wk_r[:, hc, j0:j1])
            wv_sb = wpool.tile([P, HC, H], F32R, tag="wv")
            for hc in range(HC):
                nc.gpsimd.dma_start(wv_sb[:, hc, :], wv_r[:, hc, :])

            # ---- constants (gpsimd) ----
            ident = pers.tile([P, P], F32R)
            nc.gpsimd.dma_start(ident[:], id_d)
            bqc = pers.tile([P, HC], F32)
            nc.gpsimd.dma_start(bqc[:], bq_d)
            bkc = pers.tile([P, HC], F32)
            nc.gpsimd.dma_start(bkc[:], bk_d)
            bvb = pers.tile([P, H], F32)
            nc.gpsimd.dma_start(bvb[:], bv_d)
            maskt = pers.tile([QB, NBLK, WIN], F32)
            nc.gpsimd.dma_start(maskt[:], mk_r)

            # ---- Q^T projection: [j, t] bf16, owned tokens ext [2, 1026) ----
            qt_sb = pers.tile([P, HC, SHARD], BF16)
            for t in range(2):
                for jc in range(HC):
                    ps = pproj.tile([P, 512], F32, tag="proj")
                    for hc in range(HC):
                        nc.tensor.matmul(
                            ps[:],
                            wq_sb[:, hc, jc * P : (jc + 1) * P],
                            xtb[:, hc, 2 + 512 * t : 2 + 512 * (t + 1)],
                            start=(hc == 0),
                            stop=(hc == HC - 1),
                        )
                    nc.vector.tensor_scalar_add(
                        qt_sb[:, jc, 512 * t : 512 * (t + 1)],
                        ps[:],
                        bqc[:, jc : jc + 1],
                    )

            # ---- K^T projection: [j, t] bf16, all ext tokens [0, 1028) ----
            kt_sb = pers.tile([P, HC, EXT], BF16)
            for t0, t1 in ((0, 512), (512, 1024), (1024, EXT)):
                for jc in range(HC):
                    ps = pproj.tile([P, 512], F32, tag="proj")
                    n = t1 - t0
                    for hc in range(HC):
                        nc.tensor.matmul(
                            ps[:, :n],
                            wk_sb[:, hc, jc * P : (jc + 1) * P],
                            xtb[:, hc, t0:t1],
                            start=(hc == 0),
                            stop=(hc == HC - 1),
                        )
                    nc.vector.tensor_scalar_add(
                        kt_sb[:, jc, t0:t1], ps[:, :n], bkc[:, jc : jc + 1]
                    )

            # ---- attention blocks ----
            for b in range(NBLK):
                q0 = QB * b
                qb = min(QB, SHARD - q0)
                w = qb + 2 * WCTX
                e0 = q0  # ext index of window start

                # V for this block's window, token-major [w, H], f32r
                vb = vpool.tile([P, H], F32R, tag="vblk")
                for n in range(2):
                    psv = pproj.tile([P, 512], F32, tag="proj")
                    for hc in range(HC):
                        nc.tensor.matmul(
                            psv[:w, :],
                            xt_sb[:, hc, e0 : e0 + w],
                            wv_sb[:, hc, 512 * n : 512 * (n + 1)],
                            start=(hc == 0),
                            stop=(hc == HC - 1),
                        )
                    nc.vector.tensor_copy(vb[:w, 512 * n : 512 * (n + 1)], psv[:w, :])

                # scores [qb, w] = QT_blk^T @ KT_window  (bf16 operands)
                pss = patt.tile([QB, WIN], F32, tag="ps")
                for jc in range(HC):
                    nc.tensor.matmul(
                        pss[:qb, :w],
                        qt_sb[:, jc, q0 : q0 + qb],
                        kt_sb[:, jc, e0 : e0 + w],
                        start=(jc == 0),
                        stop=(jc == HC - 1),
                    )

                # masked softmax over the window
                sm = spool.tile([QB, WIN], F32, tag="sm")
                nc.vector.tensor_tensor(
                    sm[:qb, :w], pss[:qb, :w], maskt[:qb, b, :w], op=mybir.AluOpType.add
                )
                pexp = spool.tile([QB, WIN], F32, tag="pexp")
                rsum = spool.tile([QB, 1], F32, tag="rsum")
                nc.scalar.activation(
                    pexp[:qb, :w],
                    sm[:qb, :w],
                    mybir.ActivationFunctionType.Exp,
                    bias=0.0,
                    scale=float(SCALE),
                    accum_out=rsum[:qb],
                )
                rcp = spool.tile([QB, 1], F32, tag="rcp")
                nc.vector.reciprocal(rcp[:qb], rsum[:qb])
                pn = spool.tile([QB, WIN], F32R, tag="pn")
                nc.vector.tensor_scalar_mul(pn[:qb, :w], pexp[:qb, :w], rcp[:qb])

                # P^T via PE transpose, then to SBUF
                pst = patt.tile([WIN, QB], F32R, tag="pt")
                nc.tensor.transpose(pst[:w, :qb], pn[:qb, :w], ident[:qb, :qb])
                pts = spool.tile([WIN, QB], F32R, tag="pts")
                nc.vector.tensor_copy(pts[:w, :qb], pst[:w, :qb])

                # O = P @ V  [qb, H]
                pso = pout.tile([QB, H], F32, tag="po")
                for n in range(2):
                    nc.tensor.matmul(
                        pso[:qb, 512 * n : 512 * (n + 1)],
                        pts[:w, :qb],
                        vb[:w, 512 * n : 512 * (n + 1)],
                        start=True,
                        stop=True,
                    )
                ob = opool.tile([QB, H], F32, tag="ob")
                nc.vector.tensor_tensor(
                    ob[:qb, :], pso[:qb, :], bvb[:qb, :], op=mybir.AluOpType.add
                )
                nc.sync.dma_start(out_d[q0 : q0 + qb, :], ob[:qb, :])

    nc.compile()
    return nc


def _tf32_round(a: np.ndarray) -> np.ndarray:
    u = np.ascontiguousarray(a, dtype=np.float32).view(np.uint32)
    r = (u + np.uint32(0x7FF) + ((u >> np.uint32(12)) & np.uint32(1))) & np.uint32(
        0xFFFFF000
    )
    return r.view(np.float32)


def _build_mask(h: int) -> np.ndarray:
    mask = np.full((NBLK, QB, WIN), -1e30, dtype=np.float32)
    r = np.arange(QB)[:, None]
    c = np.arange(WIN)[None, :]
    band = (c - r >= 0) & (c - r <= 2 * WCTX)
    for b in range(NBLK):
        q0 = QB * b
        qb = min(QB, SHARD - q0)
        gk = h * SHARD + q0 + c - WCTX  # global key token index
        valid = band & (gk >= 0) & (gk < S) & (r < qb) & (c < qb + 2 * WCTX)
        mask[b] = np.where(valid, np.float32(0.0), np.float32(-1e30))
    return mask


def kernel(sequence_output, Wq, bq, Wk, bk, Wv, bv):
    x = np.asarray(sequence_output, dtype=np.float32)
    Wq = np.asarray(Wq, dtype=np.float32)
    Wk = np.asarray(Wk, dtype=np.float32)
    Wv = np.asarray(Wv, dtype=np.float32)
    bq = np.asarray(bq, dtype=np.float32)
    bk = np.asarray(bk, dtype=np.float32)
    bv = np.asarray(bv, dtype=np.float32)

    if "nc" not in _prog_cache:
        _prog_cache["nc"] = _build_program()
    nc = _prog_cache["nc"]

    wq_b = Wq.astype(ml_dtypes.bfloat16)
    wk_b = Wk.astype(ml_dtypes.bfloat16)
    wv_r = _tf32_round(Wv)
    bq_c = np.ascontiguousarray(bq.reshape(HC, P).T)
    bk_c = np.ascontiguousarray(bk.reshape(HC, P).T)
    bv_b = np.ascontiguousarray(np.broadcast_to(bv, (P, H)))
    ident = np.eye(P, dtype=np.float32)
    masks = [_build_mask(0), _build_mask(1)]

    # pad each sequence with WCTX zero rows on both ends, slice ext windows
    xp = np.zeros((B, S + 2 * WCTX, H), dtype=np.float32)
    xp[:, WCTX : WCTX + S] = x

    in_maps = []
    for c in range(NCORES):
        bidx, h = divmod(c, 2)
        ext = xp[bidx, h * SHARD : h * SHARD + EXT]  # [EXT, H]
        xt = _tf32_round(np.ascontiguousarray(ext.T))  # [H, EXT]
        in_maps.append(
            {
                "xt": xt,
                "wq": wq_b,
                "wk": wk_b,
                "wv": wv_r,
                "bq_c": bq_c,
                "bk_c": bk_c,
                "bv_b": bv_b,
                "ident": ident,
                "mask": masks[h],
            }
        )

    trace = bool(int(os.environ.get("LK_TRACE", "0")))
    res = run_bass_kernel_spmd(
        nc,
        in_maps,
        core_ids=list(range(NCORES)),
        trace=trace,
        trace_cores=list(range(NCORES)) if trace else None,
    )
    _prog_cache["last_results"] = res

    out = np.empty((B, S, H), dtype=np.float32)
    for c in range(NCORES):
        bidx, h = divmod(c, 2)
        out[bidx, h * SHARD : (h + 1) * SHARD] = res.results[c]["out"]
    return out
